# revision 8
# baseline (speedup 1.0000x reference)
"""Trainium2 Bass kernel for nn_ContrastiveCRFLoss (self-contained).

Math: for each batch b and sample pair (n, m) over 2048 gathered pixels:
    out[b,n,m] = -(C[b,n,m] * (W1*exp(-cd - gd[b]/(2*BETA)) + W2*exp(-cd/(2*GAMMA))))
where C = cluster Gram, cd = squared coord distance, gd = squared guidance
distance.  The output is SYMMETRIC in (n, m): C is a Gram matrix and both
exp kernels depend on symmetric distances.

Device strategy (8 cores, upper-triangle only, mirrored on host):
  - The 2048x2048 pair grid is cut into 16 row-blocks of 128.  Block i needs
    only columns [128*i, 2048) (upper triangle incl. the diagonal block).
    Core k owns blocks k and 15-k: (16-k)+(k+1) = 17 col-units of 128 ->
    exactly 8 tiles of 256 cols + 1 tile of 128 cols per batch on EVERY
    core (SPMD-uniform shapes; per-core geometry lives in host staging).
  - Three fp16 matmul streams per tile, spread over all four PE row groups
    (tile_position 0/32/64/96) so independent streams run concurrently:
      even batches: Gram at rows 0,  exp1-arg at rows 32
      odd  batches: Gram at rows 96, exp1-arg at rows 64
      exp2-arg (batch-independent): rows 32/64/0 by chunk
    Exp arguments are produced directly in PSUM by augmenting the operands
    with ones/norm/log-weight rows (hi/lo fp16 splits keep this exact).
  - Per 1024-col chunk: ACT exp(PSUM)->fp16, add e1+e2 split GpSimd/DVE,
    DVE mult Gram(PSUM)*s -> fp16, one [128,2176] fp16 store per batch.
  - Host mirrors the strict upper triangle to reconstruct the full output.
"""

import numpy as np

import concourse.bass as bass
import concourse.mybir as mybir
import concourse.bass_utils as bass_utils
from concourse.tile import TileContext
from concourse.vector_clock import ScopedClock

F16 = mybir.dt.float16
F32 = mybir.dt.float32

# problem constants (hardcoded per the task contract)
ALPHA, BETA, GAMMA = 0.5, 0.15, 25.0
W1, W2, SHIFT = 10.0, 3.0, 0.0
B, CG, CC, H = 8, 3, 27, 224
NS = 2048
NCORES = 8
KC, K1, K2 = 27, 9, 12
NT = 9                     # column tiles per batch per core
TW = [256] * 8 + [128]     # tile widths
OFFS = [256 * t for t in range(8)] + [2048]
CORE_COLS = 2176           # sum(TW)
CHUNKS = [(0, 512), (512, 512), (1024, 512), (1536, 512), (2048, 128)]
G2 = [32, 64, 0, 96, 32]   # e2-phase row group by chunk
CHUNK_TILES = [range(0, 2), range(2, 4), range(4, 6), range(6, 8), range(8, 9)]

# ---------------------------------------------------------------------------
# Walrus in this image rejects >1 sync wait per instruction. Split the Tile
# tail-drain's waits and any multi-wait instruction into single-wait NOPs.
# ---------------------------------------------------------------------------
_MAXW = 1


def _split_drain_and_barrier(self, tick_clock, wait_clock):
    probe = self.nc.sync.nop(nofuse=True)
    wait_clock.add_sem_waits(probe.ins, ScopedClock({None: tick_clock.global_clock}))
    si = probe.ins.sync_info
    waits = list(si.on_wait)
    probe.ins.sync_info = mybir.SyncInfo(
        on_wait=waits[:_MAXW], on_update=list(si.on_update)
    )
    for i in range(_MAXW, len(waits), _MAXW):
        n2 = self.nc.sync.nop(nofuse=True)
        n2.ins.sync_info = mybir.SyncInfo(on_wait=waits[i : i + _MAXW], on_update=[])
    self.nc.sync.drain()
    self.nc.all_engine_barrier()
    popped = self.nc._tile_sem_poison_stack.pop()
    assert popped is self._sem_poison
    self.nc.clear_and_free_semaphores(list(self.sems.allocated().values()))
    self.nc.all_engine_barrier()


def _split_multiwait_insts(nc):
    n_split = 0
    for fn in nc.m.functions:
        for bb in fn.blocks:
            insts = list(bb.instructions)
            new_insts = []
            changed = False
            for inst in insts:
                si = inst.sync_info
                waits = list(si.on_wait) if si is not None else []
                if len(waits) > _MAXW:
                    n_split += 1
                    changed = True
                    n_extra = len(waits) - _MAXW
                    for i in range(0, n_extra, _MAXW):
                        nop = mybir.InstNoOp(
                            name=nc.get_next_instruction_name(),
                            engine=inst.engine,
                            bass_nofuse=True,
                            sync_info=mybir.SyncInfo(
                                on_wait=waits[i : i + _MAXW], on_update=[]
                            ),
                        )
                        new_insts.append(nop)
                    inst.sync_info = mybir.SyncInfo(
                        on_wait=waits[n_extra:], on_update=list(si.on_update)
                    )
                new_insts.append(inst)
            if changed:
                bb.instructions = new_insts
    return n_split


def _install_tile_patch():
    TileContext._drain_and_barrier = _split_drain_and_barrier


def _tiles_for_core(k):
    """17 col-units as 8x256 + 1x128 tiles: (row_block, unit_start, n_units)."""
    WA, WB = 16 - k, k + 1
    A, Bb = k, 15 - k
    tiles = []
    for j in range(0, WA - 1, 2):
        tiles.append((A, j, 2))
    for j in range(0, WB - 1, 2):
        tiles.append((Bb, j, 2))
    if WA % 2 == 0:
        tiles.append((Bb, WB - 1, 1))
    else:
        tiles.append((A, WA - 1, 1))
    assert len(tiles) == NT and sum(n for _, _, n in tiles) == 17
    return tiles


def _tiles_of_chunk(c):
    return CHUNK_TILES[c]


# ---------------------------------------------------------------------------
# Device program (identical on all cores; data differs per core)
# ---------------------------------------------------------------------------

def build_nc():
    _install_tile_patch()
    nc = bass.Bass()
    wce = nc.declare_dram_parameter("wce", [KC, 4 * NT * 128], F16, isOutput=False)
    wco = nc.declare_dram_parameter("wco", [KC, 4 * NT * 128], F16, isOutput=False)
    a1e = nc.declare_dram_parameter("a1e", [K1, 4 * NT * 128], F16, isOutput=False)
    a1o = nc.declare_dram_parameter("a1o", [K1, 4 * NT * 128], F16, isOutput=False)
    a2s = nc.declare_dram_parameter("a2s", [K2, NT * 128], F16, isOutput=False)
    rce = nc.declare_dram_parameter("rce", [KC, 4 * CORE_COLS], F16, isOutput=False)
    rco = nc.declare_dram_parameter("rco", [KC, 4 * CORE_COLS], F16, isOutput=False)
    r1e = nc.declare_dram_parameter("r1e", [K1, 4 * CORE_COLS], F16, isOutput=False)
    r1o = nc.declare_dram_parameter("r1o", [K1, 4 * CORE_COLS], F16, isOutput=False)
    r2s = nc.declare_dram_parameter("r2s", [K2, CORE_COLS], F16, isOutput=False)
    out = nc.declare_dram_parameter("out", [B, 128, CORE_COLS], F16, isOutput=True)

    WHALF = 4 * NT * 128  # 4608: per-parity lhsT slot columns
    RHALF = 4 * CORE_COLS  # 8704: per-parity rhs columns

    with TileContext(nc) as tc:
        with (
            tc.tile_pool(name="w", bufs=1) as wpool,
            tc.tile_pool(name="r", bufs=1) as rpool,
            tc.tile_pool(name="e2p", bufs=1) as e2pool,
            tc.tile_pool(name="sb", bufs=5) as sbpool,
            tc.tile_pool(name="ob", bufs=3) as opool,
            tc.tile_pool(name="ps", bufs=4, space="PSUM") as pspool,
        ):
            W = wpool.tile([128, 2 * WHALF], F16)
            R = rpool.tile([128, 2 * RHALF], F16)
            WE = wpool.tile([128, NT * 128], F16)
            RE = rpool.tile([128, CORE_COLS], F16)
            # e2 operands first (e2 phase unblocks earliest), partition
            # group per chunk: [32, 64, 0, 96, 32]
            nc.sync.dma_start(WE[32 : 32 + K2, 0:256], a2s[:, 0:256])
            nc.sync.dma_start(WE[64 : 64 + K2, 256:512], a2s[:, 256:512])
            nc.sync.dma_start(WE[0:K2, 512:768], a2s[:, 512:768])
            nc.sync.dma_start(WE[96 : 96 + K2, 768:1024], a2s[:, 768:1024])
            nc.sync.dma_start(WE[32 : 32 + K2, 1024:1152], a2s[:, 1024:1152])
            nc.sync.dma_start(RE[32 : 32 + K2, 0:512], r2s[:, 0:512])
            nc.sync.dma_start(RE[64 : 64 + K2, 512:1024], r2s[:, 512:1024])
            nc.sync.dma_start(RE[0:K2, 1024:1536], r2s[:, 1024:1536])
            nc.sync.dma_start(RE[96 : 96 + K2, 1536:2048], r2s[:, 1536:2048])
            nc.sync.dma_start(RE[32 : 32 + K2, 2048:CORE_COLS], r2s[:, 2048:CORE_COLS])
            # even-parity batch operands first (batch 0 unblocks sooner)
            nc.sync.dma_start(R[0:KC, 0:RHALF], rce[:])
            nc.sync.dma_start(R[32 : 32 + K1, 0:RHALF], r1e[:])
            nc.sync.dma_start(W[0:KC, 0:WHALF], wce[:])
            nc.sync.dma_start(W[32 : 32 + K1, 0:WHALF], a1e[:])
            nc.sync.dma_start(R[96 : 96 + KC, RHALF : 2 * RHALF], rco[:])
            nc.sync.dma_start(R[64 : 64 + K1, RHALF : 2 * RHALF], r1o[:])
            nc.sync.dma_start(W[96 : 96 + KC, WHALF : 2 * WHALF], wco[:])
            nc.sync.dma_start(W[64 : 64 + K1, WHALF : 2 * WHALF], a1o[:])

            e2 = e2pool.tile([128, CORE_COLS], F16)

            # --- e2 phase: batch-independent second-exp kernel ---
            for c, (off, wd) in enumerate(CHUNKS):
                g = G2[c]
                p2 = pspool.tile([128, 512], F32, tag="pA", name=f"p2c{c}")
                for t in _tiles_of_chunk(c):
                    w = TW[t]
                    toff = OFFS[t] - off
                    nc.tensor.matmul(
                        p2[:, toff : toff + w],
                        WE[g : g + K2, t * 128 : (t + 1) * 128],
                        RE[g : g + K2, OFFS[t] : OFFS[t] + w],
                        start=True,
                        stop=True,
                        tile_position=(g, 0),
                    )
                nc.scalar.activation(
                    e2[:, off : off + wd],
                    p2[:, 0:wd],
                    mybir.ActivationFunctionType.Exp,
                )

            # --- batch loop ---
            for b in range(B):
                par = b % 2
                gc = 0 if par == 0 else 96
                g1 = 32 if par == 0 else 64
                wbase = par * WHALF + (b // 2) * NT * 128
                rbase = par * RHALF + (b // 2) * CORE_COLS
                ob = opool.tile([128, CORE_COLS], F16, tag="ob")
                # GpSimd takes 2.5 of the 4.25 chunk-adds per batch
                gps_chunks = {0, 1, 2} if par == 0 else {0, 1}
                for c, (off, wd) in enumerate(CHUNKS):
                    p1 = pspool.tile([128, 512], F32, tag="pA", name=f"p1b{b}c{c}")
                    pC = pspool.tile([128, 512], F32, tag="pB", name=f"pCb{b}c{c}")
                    for t in _tiles_of_chunk(c):
                        w = TW[t]
                        toff = OFFS[t] - off
                        nc.tensor.matmul(
                            p1[:, toff : toff + w],
                            W[g1 : g1 + K1, wbase + t * 128 : wbase + (t + 1) * 128],
                            R[g1 : g1 + K1, rbase + OFFS[t] : rbase + OFFS[t] + w],
                            start=True,
                            stop=True,
                            tile_position=(g1, 0),
                        )
                        nc.tensor.matmul(
                            pC[:, toff : toff + w],
                            W[gc : gc + KC, wbase + t * 128 : wbase + (t + 1) * 128],
                            R[gc : gc + KC, rbase + OFFS[t] : rbase + OFFS[t] + w],
                            start=True,
                            stop=True,
                            tile_position=(gc, 0),
                        )
                    e1 = sbpool.tile([128, 512], F16, tag="e1")
                    s = sbpool.tile([128, 512], F16, tag="s")
                    nc.scalar.activation(
                        e1[:, 0:wd],
                        p1[:, 0:wd],
                        mybir.ActivationFunctionType.Exp,
                    )
                    if c in gps_chunks:
                        nc.gpsimd.tensor_add(
                            s[:, 0:wd], e1[:, 0:wd], e2[:, off : off + wd]
                        )
                    else:
                        nc.vector.tensor_add(
                            s[:, 0:wd], e1[:, 0:wd], e2[:, off : off + wd]
                        )
                    nc.vector.tensor_tensor(
                        ob[:, off : off + wd],
                        pC[:, 0:wd],
                        s[:, 0:wd],
                        mybir.AluOpType.mult,
                    )
                nc.sync.dma_start(out[b], ob[:])

    _split_multiwait_insts(nc)
    return nc


# ---------------------------------------------------------------------------
# Host-side input prep
# ---------------------------------------------------------------------------

def _f16(x):
    return np.asarray(x, dtype=np.float16)


def _hi_lo(x):
    """Split fp64 vector into two fp16 rows summing to ~x."""
    hi = _f16(x)
    lo = _f16(x - hi.astype(np.float64))
    return hi, lo


def prepare_inputs(guidance, clusters, coords):
    ci = np.asarray(coords[0], dtype=np.int64)
    cj = np.asarray(coords[1], dtype=np.int64)
    sel_g = guidance[:, :, ci, cj].astype(np.float64)  # [B, 3, NS]
    sel_c = clusters[:, :, ci, cj].astype(np.float32)  # [B, 27, NS]

    # --- cluster Gram operands (fp16 snap) ---
    c16 = _f16(sel_c)
    wc_all = -c16  # lhsT (negated -> folds the leading minus)

    # --- first-exp argument operands: arg1 = -cd - gd/(2*beta) + ln(W1) ---
    u16 = _f16(sel_g / np.sqrt(2.0 * BETA))  # [B, 3, NS]
    xc16 = _f16(np.stack([ci, cj]) - 112.0)  # [2, NS] exact
    f1 = (u16.astype(np.float64) ** 2).sum(1) + (
        xc16.astype(np.float64) ** 2
    ).sum(0)  # [B, NS]
    a1_all = np.empty((B, K1, NS), np.float16)
    r1_all = np.empty((B, K1, NS), np.float16)
    ones = np.ones(NS, np.float16)
    for b in range(B):
        b1h, b1l = _hi_lo(np.log(W1) - f1[b])
        f1h, f1l = _hi_lo(f1[b])
        a1_all[b, 0:3] = u16[b]
        a1_all[b, 3:5] = xc16
        a1_all[b, 5] = ones
        a1_all[b, 6] = ones
        a1_all[b, 7] = f1h
        a1_all[b, 8] = f1l
        r1_all[b, 0:3] = _f16(2.0 * u16[b].astype(np.float64))
        r1_all[b, 3:5] = _f16(2.0 * xc16.astype(np.float64))
        r1_all[b, 5] = b1h
        r1_all[b, 6] = b1l
        r1_all[b, 7] = -ones
        r1_all[b, 8] = -ones

    # --- second-exp argument operands (batch independent) ---
    v = (np.stack([ci, cj]) - 112.0) / np.sqrt(2.0 * GAMMA)  # [2, NS]
    vh = _f16(v)
    vl = _f16(v - vh.astype(np.float64))
    vs = vh.astype(np.float64) + vl.astype(np.float64)
    f2 = (vs**2).sum(0)
    b2h, b2l = _hi_lo(np.log(W2) - f2)
    f2h, f2l = _hi_lo(f2)
    a2 = np.empty((K2, NS), np.float16)
    r2 = np.empty((K2, NS), np.float16)
    a2[0:2] = vh
    a2[2:4] = vh
    a2[4:6] = vl
    a2[6:8] = vl
    r2[0:2] = _f16(2.0 * vh.astype(np.float64))
    r2[2:4] = _f16(2.0 * vl.astype(np.float64))
    r2[4:6] = _f16(2.0 * vh.astype(np.float64))
    r2[6:8] = _f16(2.0 * vl.astype(np.float64))
    a2[8] = ones
    a2[9] = ones
    a2[10] = f2h
    a2[11] = f2l
    r2[8] = b2h
    r2[9] = b2l
    r2[10] = -ones
    r2[11] = -ones

    in_maps = []
    for k in range(NCORES):
        tiles = _tiles_for_core(k)
        wce = np.empty((KC, 4 * NT * 128), np.float16)
        wco = np.empty((KC, 4 * NT * 128), np.float16)
        a1e = np.empty((K1, 4 * NT * 128), np.float16)
        a1o = np.empty((K1, 4 * NT * 128), np.float16)
        a2sk = np.empty((K2, NT * 128), np.float16)
        rce = np.empty((KC, 4 * CORE_COLS), np.float16)
        rco = np.empty((KC, 4 * CORE_COLS), np.float16)
        r1e = np.empty((K1, 4 * CORE_COLS), np.float16)
        r1o = np.empty((K1, 4 * CORE_COLS), np.float16)
        r2sk = np.empty((K2, CORE_COLS), np.float16)
        for t, (X, j, n) in enumerate(tiles):
            rows = slice(128 * X, 128 * X + 128)
            cols = slice(128 * (X + j), 128 * (X + j) + 128 * n)
            o_t, w = OFFS[t], TW[t]
            a2sk[:, t * 128 : (t + 1) * 128] = a2[:, rows]
            r2sk[:, o_t : o_t + w] = r2[:, cols]
            for bi in range(4):
                for par, (wc_d, a1_d, rc_d, r1_d) in enumerate(
                    [(wce, a1e, rce, r1e), (wco, a1o, rco, r1o)]
                ):
                    b = 2 * bi + par
                    sl = slice((bi * NT + t) * 128, (bi * NT + t + 1) * 128)
                    cl = slice(bi * CORE_COLS + o_t, bi * CORE_COLS + o_t + w)
                    wc_d[:, sl] = wc_all[b][:, rows]
                    a1_d[:, sl] = a1_all[b][:, rows]
                    rc_d[:, cl] = c16[b][:, cols]
                    r1_d[:, cl] = r1_all[b][:, cols]
        in_maps.append(
            {
                "wce": wce, "wco": wco, "a1e": a1e, "a1o": a1o, "a2s": a2sk,
                "rce": rce, "rco": rco, "r1e": r1e, "r1o": r1o, "r2s": r2sk,
            }
        )
    return in_maps


_NC_CACHE = {}


def _get_nc():
    if "nc" not in _NC_CACHE:
        _NC_CACHE["nc"] = build_nc()
    return _NC_CACHE["nc"]


def kernel(guidance, clusters, coords):
    guidance = np.asarray(guidance)
    clusters = np.asarray(clusters)
    coords = np.asarray(coords)
    in_maps = prepare_inputs(guidance, clusters, coords)
    nc = _get_nc()
    res = bass_utils.run_bass_kernel_spmd(nc, in_maps, list(range(NCORES)))
    # reassemble upper triangle, then mirror
    full = np.zeros((B, NS, NS), np.float32)
    for k in range(NCORES):
        o = res.results[k]["out"].astype(np.float32)  # [B, 128, CORE_COLS]
        for t, (X, j, n) in enumerate(_tiles_for_core(k)):
            rows = slice(128 * X, 128 * X + 128)
            cols = slice(128 * (X + j), 128 * (X + j) + 128 * n)
            full[:, rows, cols] = o[:, :, OFFS[t] : OFFS[t] + TW[t]]
    up = np.triu(full, 1)
    full = np.triu(full) + np.swapaxes(up, 1, 2)
    return full
